# revision 26
# baseline (speedup 1.0000x reference)
"""Trainium2 Bass kernel for nn_Block_78993038508729 (dense transformer
block: rmsnorm -> causal MHA (+degenerate rope) -> rmsnorm -> top-2 MoE
with SwiGLU experts).

Strategy (8 NeuronCores):
  Launch A: attention, tensor-parallel over heads (2 heads/core). Each core
    computes rmsnorm(x), projects its q/k/v head slices, runs causal
    softmax attention, and emits its partial contribution of y @ wo.
    Host sums the 8 partials (TP unshard) and adds the residual.
  Host: rmsnorm2 + router + exact top-2 + per-expert token gather
    (routing is data-dependent; gather/scatter is host-side unshard work).
  Launch B: experts, expert-parallel (expert e on core e). Each core runs
    silu(tok@gate)*(tok@up) @ down for its expert's tokens, scaled by the
    routing weight. Host scatter-adds results (exact: non-selected experts
    have weight exactly 0 in the reference).

Note on rope: the reference's rope slices freqs[:NH] and broadcasts over
the sequence axis, so the rotation for each head is constant across
positions and identical for q and k. A fixed rotation applied to both
operands of a dot product cancels (orthogonal transform), so attention
scores -- and therefore the block output -- are unchanged by skipping it.

Attention matmuls use float32r (TF32-like PE mode: 1 cycle/row at N>=256,
~2^-11 relative precision -- keeps router logits accurate so the top-2
expert choice matches the reference). The MoE runs in bf16 (fast weight
loads; routing is already decided so precision only affects the expert
outputs, which are small relative to the residual).

Hardware constraint found empirically: PE transposes with different base
partitions must not be interleaved into the same PSUM tile -- the PE
wedges (NRT_EXEC_UNIT_UNRECOVERABLE). Transposes are grouped per base.
"""

import sys

if "/opt/trn_rl_repo" not in sys.path:
    sys.path.insert(0, "/opt/trn_rl_repo")

import math

import ml_dtypes
import numpy as np

import concourse.bass as bass
import concourse.mybir as mybir
import concourse.tile as tile
from concourse import bacc
from concourse.bass_utils import run_bass_kernel_spmd

F32 = mybir.dt.float32
F32R = mybir.dt.float32r
BF16 = mybir.dt.bfloat16
AF = mybir.ActivationFunctionType
BF16_NP = ml_dtypes.bfloat16

B, T, D = 1, 2048, 1024
NH, HD = 16, 64
E, K, H = 8, 2, 2048
LAYER_DEPTH = 12
EPS = 1e-8
NCORES = 8
HPC = NH // NCORES          # heads per core = 2
CW = HPC * HD               # per-core head-column width = 128
CAP = 640                   # token capacity per expert core (launch B)
MOE_SCALE = 1.0 / math.sqrt(LAYER_DEPTH)

_CACHE: dict = {}

# CoreSim doesn't implement the Silu activation; when True, build
# silu(g) as g*sigmoid(g) (identical formula, LUT-free path).
SIM_COMPAT = False


def _bacc(n_cores):
    return bacc.Bacc("TRN2", target_bir_lowering=False, debug=False,
                     num_devices=n_cores)


# --------------------------------------------------------------------------
# Launch A: attention (head-sharded).
# Per-core inputs:
#   x      [T, D]   f32   full input (replicated)
#   wqkv   [D, 3*CW] f32r  [wq_c | wk_c | wv_c] columns for this core's heads
#   bqkv   [3, CW]  f32   row 0 bq_c, row 1 bk_c, row 2 bv_c
#   wo     [CW, D]  f32r  wo rows for this core's head columns
#   trimask [128, 128] f32  triu mask: m[tk, u] = 1 iff u >= tk
#   ident  [128, 128] f32r  identity (PE transpose)
#   ident2 [128, 64] f32r  eye(64) stacked twice (per-head transposes)
#   onesc  [1, 128]  f32r  ones row (K=1 broadcast matmuls)
# Output:
#   part   [T, D]   f32   this core's partial of y @ wo (normalized)
# --------------------------------------------------------------------------

def build_attn():
    nc = _bacc(NCORES)
    x_d = nc.dram_tensor("x", [T, D], F32, kind="ExternalInput")
    wqkv_d = nc.dram_tensor("wqkv", [D, 3 * CW], F32R, kind="ExternalInput")
    bqkv_d = nc.dram_tensor("bqkv", [3, CW], F32, kind="ExternalInput")
    wo_d = nc.dram_tensor("wo", [CW, D], F32R, kind="ExternalInput")
    trimask_d = nc.dram_tensor("trimask", [128, 128], F32, kind="ExternalInput")
    ident_d = nc.dram_tensor("ident", [128, 128], F32R, kind="ExternalInput")
    ident2_d = nc.dram_tensor("ident2", [128, 64], F32R, kind="ExternalInput")
    onesc_d = nc.dram_tensor("onesc", [1, 128], F32R, kind="ExternalInput")
    part_d = nc.dram_tensor("part", [T, D], F32, kind="ExternalOutput")

    NT = T // 128            # token tiles
    NJ = T // 512            # big token blocks
    NC = D // 128            # contraction chunks

    with tile.TileContext(nc, num_cores=NCORES) as tc:
        with (
            tc.tile_pool(name="const", bufs=1) as const,
            tc.tile_pool(name="xin", bufs=2) as xin,
            tc.tile_pool(name="stat", bufs=4) as stat,
            tc.tile_pool(name="ht", bufs=1) as htp,
            tc.tile_pool(name="qkv", bufs=1) as qkvp,
            tc.tile_pool(name="expp", bufs=6) as expp,
            tc.tile_pool(name="yout", bufs=4) as youtp,
            tc.tile_pool(name="ps", bufs=2, space="PSUM") as ps,
            tc.tile_pool(name="psy", bufs=2, space="PSUM") as psy,
        ):
            ident = const.tile([128, 128], F32R)
            nc.sync.dma_start(out=ident[:], in_=ident_d[:, :])
            ident2 = const.tile([128, 64], F32R)
            nc.sync.dma_start(out=ident2[:], in_=ident2_d[:, :])
            onesc = const.tile([1, 128], F32R)
            nc.sync.dma_start(out=onesc[:], in_=onesc_d[:, :])
            trimask = const.tile([128, 128], F32)
            nc.sync.dma_start(out=trimask[:], in_=trimask_d[:, :])
            wqkv = const.tile([128, NC, 3 * CW], F32R)
            nc.sync.dma_start(
                out=wqkv[:], in_=wqkv_d.ap().rearrange("(c p) m -> p c m", p=128))
            bqkv = const.tile([128, 3], F32)
            nc.sync.dma_start(
                out=bqkv[:], in_=bqkv_d.ap().rearrange("r m -> m r"))
            wo = const.tile([128, D], F32R)
            nc.sync.dma_start(out=wo[:], in_=wo_d[:, :])
            ones16 = const.tile([128, NT], F32)
            nc.vector.memset(ones16[:], 1.0)
            epst = const.tile([128, 1], F32)
            nc.vector.memset(epst[:], EPS)

            hT = htp.tile([128, NC, T], F32R)
            qT = qkvp.tile([128, T], F32R)
            kT = qkvp.tile([128, T], F32R)
            vT = qkvp.tile([128, T], F32R)
            vext = [youtp.tile([128, NT, HD + 1], F32R, name=f"vext{h}", bufs=1)
                    for h in range(HPC)]
            yT = qkvp.tile([128, T], F32R)
            dens = [qkvp.tile([1, T], F32R, name=f"den{h}") for h in range(HPC)]

            for h in range(HPC):
                nc.scalar.activation(out=vext[h][:, :, HD], in_=ones16[:],
                                     func=AF.Copy)

            def phase123(j):
                """rmsnorm + transpose + qkv projection + v transpose for
                token block j (512 tokens = 4 tiles)."""
                jsl = bass.ts(j, 512)
                for i in range(4 * j, 4 * j + 4):
                    xt = xin.tile([128, D], F32)
                    nc.sync.dma_start(out=xt[:],
                                      in_=x_d[i * 128:(i + 1) * 128, :])
                    ssum = stat.tile([128, 1], F32)
                    sq = stat.tile([128, D], F32, bufs=2)
                    nc.gpsimd.tensor_mul(sq[:], xt[:], xt[:])
                    nc.vector.reduce_sum(ssum[:], sq[:],
                                         axis=mybir.AxisListType.X)
                    rstd = stat.tile([128, 1], F32)
                    nc.scalar.activation(out=rstd[:], in_=ssum[:],
                                         func=AF.Sqrt, scale=1.0 / D,
                                         bias=epst[:])
                    nc.vector.reciprocal(out=rstd[:], in_=rstd[:])
                    h = xin.tile([128, D], F32R)
                    nc.vector.tensor_scalar_mul(h[:], xt[:], rstd[:])
                    for c2 in range(0, NC, 4):
                        ptr = ps.tile([128, 512], F32R, tag="tr")
                        for c in range(c2, c2 + 4):
                            nc.tensor.transpose(
                                ptr[:, (c - c2) * 128:(c - c2 + 1) * 128],
                                h[:, c * 128:(c + 1) * 128], ident[:])
                        dst = hT[:, c2:c2 + 4, i * 128:(i + 1) * 128]
                        src = ptr[:].rearrange("p (c q) -> p c q", q=128)
                        if (i + c2) % 2 == 0:
                            nc.scalar.copy(dst, src)
                        else:
                            nc.vector.tensor_copy(dst, src)
                # qkv projections for this block
                for out_t, col0, brow in ((qT, 0, 0), (kT, CW, 1),
                                          (vT, 2 * CW, 2)):
                    pmm = psy.tile([128, 512], F32, tag="yacc")
                    for c in range(NC):
                        nc.tensor.matmul(
                            pmm[:], wqkv[:, c, col0:col0 + CW],
                            hT[:, c, jsl], start=(c == 0), stop=(c == NC - 1))
                    if brow < 2:
                        nc.vector.tensor_scalar_add(out_t[:, jsl], pmm[:],
                                                    bqkv[:, brow:brow + 1])
                    else:
                        nc.scalar.activation(out=out_t[:, jsl], in_=pmm[:],
                                             func=AF.Identity,
                                             bias=bqkv[:, brow:brow + 1])
                # v transposes for this block (grouped per head: PE wedges on
                # mixed-base transposes within one psum tile)
                for h in range(HPC):
                    ptr = ps.tile([128, 256], F32R, tag="trv")
                    for i in range(4 * j, 4 * j + 4):
                        slot = (i - 4 * j) * 64
                        nc.tensor.transpose(
                            ptr[:, slot:slot + 64],
                            vT[h * HD:(h + 1) * HD, i * 128:(i + 1) * 128],
                            ident2[h * HD:(h + 1) * HD, :])
                    nc.vector.tensor_copy(
                        vext[h][:, 4 * j:4 * j + 4, 0:HD],
                        ptr[:].rearrange("p (i d) -> p i d", d=64))

            def attention(jq):
                """causal attention for query block jq, both heads."""
                jsl = bass.ts(jq, 512)
                for h in range(HPC):
                    hsl = slice(h * HD, (h + 1) * HD)
                    pacc = psy.tile([128, 512], F32, tag="yacc")
                    nblk = 4 * jq + 4
                    for ib in range(nblk):
                        off = (ib - 4 * jq) * 128 if ib >= 4 * jq else 0
                        pss = ps.tile([128, 512], F32, tag="scores")
                        nc.tensor.matmul(
                            pss[:, off:512],
                            kT[hsl, ib * 128:(ib + 1) * 128],
                            qT[hsl, jsl][:, off:512], start=True, stop=True)
                        et = expp.tile([128, 512], F32R, tag="exp")
                        nc.scalar.activation(out=et[:, off:512],
                                             in_=pss[:, off:512],
                                             func=AF.Exp,
                                             scale=1.0 / math.sqrt(HD))
                        if ib >= 4 * jq:
                            # triangular boundary strip
                            nc.vector.tensor_mul(et[:, off:off + 128],
                                                 et[:, off:off + 128],
                                                 trimask[:])
                        nc.tensor.matmul(
                            pacc[0:HD + 1, off:512], vext[h][:, ib, :],
                            et[:, off:512],
                            start=(ib == 0), stop=(ib == nblk - 1))
                    if h == 0:
                        nc.scalar.copy(yT[hsl, jsl], pacc[0:HD, :])
                    else:
                        nc.vector.tensor_copy(yT[hsl, jsl], pacc[0:HD, :])
                    nc.vector.tensor_copy(dens[h][0:1, jsl],
                                          pacc[HD:HD + 1, :])

            for j in range(NJ):
                phase123(j)
                attention(j)

            # ---- normalize yT by denominators ----
            with nc.allow_low_precision(
                    reason="f32r rounding of softmax denominator "
                           "reciprocals (~2^-11) is negligible"):
                for h in range(HPC):
                    nc.vector.reciprocal(out=dens[h][:], in_=dens[h][:])
            for h in range(HPC):
                hsl = slice(h * HD, (h + 1) * HD)
                for j in range(NJ):
                    jsl = bass.ts(j, 512)
                    pbd = psy.tile([128, 512], F32, tag="yacc")
                    nc.tensor.matmul(pbd[:], onesc[:], dens[h][0:1, jsl],
                                     start=True, stop=True)
                    nc.vector.tensor_mul(yT[hsl, jsl], yT[hsl, jsl],
                                         pbd[hsl, :])

            # ---- partial output projection: part = yTn.T @ wo ----
            for i in range(NT):
                for half in range(2):
                    pso = psy.tile([128, 512], F32, tag="yacc")
                    nc.tensor.matmul(
                        pso[:], yT[:, i * 128:(i + 1) * 128],
                        wo[:, half * 512:(half + 1) * 512],
                        start=True, stop=True)
                    ot = youtp.tile([128, 512], F32, tag="out")
                    if (i + half) % 2 == 0:
                        nc.scalar.copy(ot[:], pso[:])
                    else:
                        nc.vector.tensor_copy(ot[:], pso[:])
                    nc.sync.dma_start(
                        out=part_d[i * 128:(i + 1) * 128,
                                   half * 512:(half + 1) * 512],
                        in_=ot[:])
    nc.compile()
    return nc


# --------------------------------------------------------------------------
# Launch B: one expert per core (bf16 matmuls, fp32 accumulation).
# Per-core inputs:
#   tokT [D, CAP]  bf16  gathered+normed tokens (transposed), zero-padded
#   gu   [D, 2H]   bf16  [gate | up] for this core's expert
#   down [H, D]    bf16  down projection
#   wts  [CAP/128, 128] f32  routing weight * MOE_SCALE per slot (0 for pads)
# Output:
#   eout [CAP, D]  f32   weighted expert output per slot
# --------------------------------------------------------------------------

def build_moe():
    nc = _bacc(NCORES)
    tokT_d = nc.dram_tensor("tokT", [D, CAP], BF16, kind="ExternalInput")
    gu_d = nc.dram_tensor("gu", [D, 2 * H], BF16, kind="ExternalInput")
    down_d = nc.dram_tensor("down", [H, D], BF16, kind="ExternalInput")
    wts_d = nc.dram_tensor("wts", [CAP // 128, 128], F32, kind="ExternalInput")
    eout_d = nc.dram_tensor("eout", [CAP, D], F32, kind="ExternalOutput")

    NC = D // 128            # 8 d chunks
    NHT = H // 128           # 16 h tiles
    NTT = CAP // 128         # 5 token tiles

    with tile.TileContext(nc, num_cores=NCORES) as tc:
        with (
            tc.tile_pool(name="const", bufs=1) as const,
            tc.tile_pool(name="wstream", bufs=3) as wstream,
            tc.tile_pool(name="gup", bufs=1) as gup,
            tc.tile_pool(name="outp", bufs=4) as outp,
            tc.tile_pool(name="ps", bufs=2, space="PSUM") as ps,
            tc.tile_pool(name="psu", bufs=2, space="PSUM") as psu,
        ):
            tokT = const.tile([128, NC, CAP], BF16)
            nc.sync.dma_start(
                out=tokT[:], in_=tokT_d.ap().rearrange("(c p) n -> p c n", p=128))
            wts = const.tile([128, NTT], F32)
            nc.sync.dma_start(out=wts[:], in_=wts_d.ap().rearrange("t p -> p t"))
            down = const.tile([128, NHT, D], BF16)
            nc.sync.dma_start(
                out=down[:], in_=down_d.ap().rearrange("(t p) m -> p t m", p=128))

            guT = gup.tile([128, NHT, CAP], BF16)
            for t in range(NHT):
                gw = wstream.tile([128, NC, 128], BF16, tag="gw")
                nc.sync.dma_start(
                    out=gw[:],
                    in_=gu_d.ap()[:, t * 128:(t + 1) * 128]
                    .rearrange("(c p) m -> p c m", p=128))
                uw = wstream.tile([128, NC, 128], BF16, tag="uw")
                nc.sync.dma_start(
                    out=uw[:],
                    in_=gu_d.ap()[:, H + t * 128:H + (t + 1) * 128]
                    .rearrange("(c p) m -> p c m", p=128))
                for n0, n1 in ((0, 512), (512, CAP)):
                    psg = ps.tile([128, 512], F32, tag="g")
                    psuu = psu.tile([128, 512], F32, tag="u")
                    nw = n1 - n0
                    for c in range(NC):
                        nc.tensor.matmul(psg[:, 0:nw], gw[:, c, :],
                                         tokT[:, c, n0:n1],
                                         start=(c == 0), stop=(c == NC - 1))
                    for c in range(NC):
                        nc.tensor.matmul(psuu[:, 0:nw], uw[:, c, :],
                                         tokT[:, c, n0:n1],
                                         start=(c == 0), stop=(c == NC - 1))
                    sg = outp.tile([128, 512], F32, tag="sg")
                    if SIM_COMPAT:
                        nc.scalar.activation(out=sg[:, 0:nw], in_=psg[:, 0:nw],
                                             func=AF.Sigmoid)
                        nc.vector.tensor_mul(sg[:, 0:nw], sg[:, 0:nw],
                                             psg[:, 0:nw])
                    else:
                        nc.scalar.activation(out=sg[:, 0:nw], in_=psg[:, 0:nw],
                                             func=AF.Silu)
                    nc.vector.tensor_mul(guT[:, t, n0:n1], sg[:, 0:nw],
                                         psuu[:, 0:nw])

            for tt in range(NTT):
                for half in range(2):
                    pso = ps.tile([128, 512], F32, tag="o")
                    for t in range(NHT):
                        nc.tensor.matmul(
                            pso[:], guT[:, t, tt * 128:(tt + 1) * 128],
                            down[:, t, half * 512:(half + 1) * 512],
                            start=(t == 0), stop=(t == NHT - 1))
                    ot = outp.tile([128, 512], F32, tag="ot")
                    nc.vector.tensor_scalar_mul(ot[:], pso[:],
                                                wts[:, tt:tt + 1])
                    nc.sync.dma_start(
                        out=eout_d[tt * 128:(tt + 1) * 128,
                                   half * 512:(half + 1) * 512],
                        in_=ot[:])
    nc.compile()
    return nc


# --------------------------------------------------------------------------
# Host orchestration
# --------------------------------------------------------------------------

def _get(name, builder):
    if name not in _CACHE:
        _CACHE[name] = builder()
    return _CACHE[name]


def _attn_inputs(x2d, wq, bq, wkv, bkv, wo, norm1_w):
    """Build the 8 per-core input maps for launch A."""
    # fold norm1_w into the projection rows
    wq_s = wq * norm1_w[:, None]
    wkv_s = wkv * norm1_w[:, None]
    wk_s = wkv_s[:, :D]
    wv_s = wkv_s[:, D:]
    bk = bkv[:D]
    bv = bkv[D:]

    tk = np.arange(128)[:, None]
    u = np.arange(128)[None, :]
    trimask = (u >= tk).astype(np.float32)
    ident = np.eye(128, dtype=np.float32)
    ident2 = np.concatenate([np.eye(64, dtype=np.float32)] * 2, axis=0)
    onesc = np.ones((1, 128), np.float32)

    ins = []
    for c in range(NCORES):
        cs = slice(c * CW, (c + 1) * CW)
        wqkv_c = np.ascontiguousarray(
            np.concatenate([wq_s[:, cs], wk_s[:, cs], wv_s[:, cs]], axis=1))
        bqkv_c = np.ascontiguousarray(
            np.stack([bq[cs], bk[cs], bv[cs]], axis=0))
        wo_c = np.ascontiguousarray(wo[cs, :])
        ins.append({
            "x": x2d,
            "wqkv": wqkv_c,
            "bqkv": bqkv_c,
            "wo": wo_c,
            "trimask": trimask,
            "ident": ident,
            "ident2": ident2,
            "onesc": onesc,
        })
    return ins


def _route(x2, router_w, norm2_w):
    """Exact reference routing on host: rmsnorm2 + top-2 + softmax."""
    h2 = x2 / np.sqrt(np.mean(x2 * x2, axis=-1, keepdims=True) + EPS)
    h2 = (h2 * norm2_w).astype(np.float32)
    logits = h2.astype(np.float32) @ router_w.astype(np.float32)   # [N, E]
    idx1 = np.argmax(logits, axis=-1)
    l2 = logits.copy()
    l2[np.arange(T), idx1] = -np.inf
    idx2 = np.argmax(l2, axis=-1)
    v1 = logits[np.arange(T), idx1]
    v2 = logits[np.arange(T), idx2]
    # softmax over the two selected logits (v1 >= v2)
    e2 = np.exp((v2 - v1).astype(np.float32))
    p1 = (1.0 / (1.0 + e2)).astype(np.float32)
    p2 = (e2 / (1.0 + e2)).astype(np.float32)
    return h2, idx1, idx2, p1, p2


def kernel(x, freqs_cos, freqs_sin, norm1_w, wq, bq, wkv, bkv, wo, bo,
           norm2_w, router_w, gate_w, up_w, down_w):
    x = np.asarray(x, np.float32)
    x2d = np.ascontiguousarray(x.reshape(T, D))
    wq = np.asarray(wq, np.float32)
    wkv = np.asarray(wkv, np.float32)
    wo = np.asarray(wo, np.float32)
    bq = np.asarray(bq, np.float32)
    bkv = np.asarray(bkv, np.float32)
    bo = np.asarray(bo, np.float32)
    norm1_w = np.asarray(norm1_w, np.float32)
    norm2_w = np.asarray(norm2_w, np.float32)
    router_w = np.asarray(router_w, np.float32)
    gate_w = np.asarray(gate_w, np.float32)
    up_w = np.asarray(up_w, np.float32)
    down_w = np.asarray(down_w, np.float32)

    # ---- launch A ----
    nc_a = _get("attn", build_attn)
    ins_a = _attn_inputs(x2d, wq, bq, wkv, bkv, wo, norm1_w)
    res_a = run_bass_kernel_spmd(nc_a, ins_a, core_ids=list(range(NCORES)))
    parts = np.stack([res_a.results[c]["part"] for c in range(NCORES)])
    x2 = (x2d.astype(np.float64) + parts.sum(axis=0, dtype=np.float64)
          + bo.astype(np.float64)).astype(np.float32)

    # ---- host routing ----
    h2, idx1, idx2, p1, p2 = _route(x2, router_w, norm2_w)

    # per-expert token lists (order: top-1 hits then top-2 hits, stable)
    work = []   # (expert, token_idx array, weight array)
    for e in range(E):
        m1 = idx1 == e
        m2 = idx2 == e
        toks = np.concatenate([np.nonzero(m1)[0], np.nonzero(m2)[0]])
        wgts = np.concatenate([p1[m1], p2[m2]]).astype(np.float32)
        for s in range(0, len(toks), CAP):
            work.append((e, toks[s:s + CAP], wgts[s:s + CAP]))

    h2b = h2.astype(BF16_NP)
    gub: dict = {}
    downb: dict = {}

    # ---- launch B (usually one round of 8) ----
    nc_b = _get("moe", build_moe)
    moe = np.zeros((T, D), np.float64)
    for r0 in range(0, len(work), NCORES):
        batch = work[r0:r0 + NCORES]
        while len(batch) < NCORES:
            batch.append((0, np.zeros(0, np.int64), np.zeros(0, np.float32)))
        ins_b = []
        for e, toks, wgts in batch:
            tokT = np.zeros((D, CAP), BF16_NP)
            tokT[:, :len(toks)] = h2b[toks].T
            wts = np.zeros((CAP,), np.float32)
            wts[:len(toks)] = wgts * MOE_SCALE
            if e not in gub:
                gub[e] = np.ascontiguousarray(np.concatenate(
                    [gate_w[e], up_w[e]], axis=1).astype(BF16_NP))
                downb[e] = np.ascontiguousarray(down_w[e].astype(BF16_NP))
            ins_b.append({
                "tokT": tokT,
                "gu": gub[e],
                "down": downb[e],
                "wts": np.ascontiguousarray(wts.reshape(CAP // 128, 128)),
            })
        res_b = run_bass_kernel_spmd(nc_b, ins_b, core_ids=list(range(NCORES)))
        for (e, toks, wgts), rc in zip(batch, res_b.results):
            if len(toks):
                moe[toks] += rc["eout"][:len(toks)].astype(np.float64)

    out = (x2.astype(np.float64) + moe).astype(np.float32)
    return out.reshape(B, T, D)


# revision 27
# speedup vs baseline: 1.0422x; 1.0422x over previous
"""Trainium2 Bass kernel for nn_Block_78993038508729 (dense transformer
block: rmsnorm -> causal MHA (+degenerate rope) -> rmsnorm -> top-2 MoE
with SwiGLU experts).

Strategy (8 NeuronCores):
  Launch A: attention, tensor-parallel over heads (2 heads/core). Each core
    computes rmsnorm(x), projects its q/k/v head slices, runs causal
    softmax attention, and emits its partial contribution of y @ wo.
    Host sums the 8 partials (TP unshard) and adds the residual.
  Host: rmsnorm2 + router + exact top-2 + per-expert token gather
    (routing is data-dependent; gather/scatter is host-side unshard work).
  Launch B: experts, expert-parallel (expert e on core e). Each core runs
    silu(tok@gate)*(tok@up) @ down for its expert's tokens, scaled by the
    routing weight. Host scatter-adds results (exact: non-selected experts
    have weight exactly 0 in the reference).

Note on rope: the reference's rope slices freqs[:NH] and broadcasts over
the sequence axis, so the rotation for each head is constant across
positions and identical for q and k. A fixed rotation applied to both
operands of a dot product cancels (orthogonal transform), so attention
scores -- and therefore the block output -- are unchanged by skipping it.

Attention matmuls use float32r (TF32-like PE mode: 1 cycle/row at N>=256,
~2^-11 relative precision -- keeps router logits accurate so the top-2
expert choice matches the reference). The MoE runs in bf16 (fast weight
loads; routing is already decided so precision only affects the expert
outputs, which are small relative to the residual).

Hardware constraint found empirically: PE transposes with different base
partitions must not be interleaved into the same PSUM tile -- the PE
wedges (NRT_EXEC_UNIT_UNRECOVERABLE). Transposes are grouped per base.
"""

import sys

if "/opt/trn_rl_repo" not in sys.path:
    sys.path.insert(0, "/opt/trn_rl_repo")

import math

import ml_dtypes
import numpy as np

import concourse.bass as bass
import concourse.mybir as mybir
import concourse.tile as tile
from concourse import bacc
from concourse.bass_utils import run_bass_kernel_spmd

F32 = mybir.dt.float32
F32R = mybir.dt.float32r
BF16 = mybir.dt.bfloat16
AF = mybir.ActivationFunctionType
BF16_NP = ml_dtypes.bfloat16

B, T, D = 1, 2048, 1024
NH, HD = 16, 64
E, K, H = 8, 2, 2048
LAYER_DEPTH = 12
EPS = 1e-8
NCORES = 8
HPC = NH // NCORES          # heads per core = 2
CW = HPC * HD               # per-core head-column width = 128
CAP = 640                   # token capacity per expert core (launch B)
MOE_SCALE = 1.0 / math.sqrt(LAYER_DEPTH)

_CACHE: dict = {}

# CoreSim doesn't implement the Silu activation; when True, build
# silu(g) as g*sigmoid(g) (identical formula, LUT-free path).
SIM_COMPAT = False


def _bacc(n_cores):
    return bacc.Bacc("TRN2", target_bir_lowering=False, debug=False,
                     num_devices=n_cores)


# --------------------------------------------------------------------------
# Launch A: attention (head-sharded).
# Per-core inputs:
#   x      [T, D]   f32   full input (replicated)
#   wqkv   [D, 3*CW] f32r  [wq_c | wk_c | wv_c] columns for this core's heads
#   bqkv   [3, CW]  f32   row 0 bq_c, row 1 bk_c, row 2 bv_c
#   wo     [CW, D]  f32r  wo rows for this core's head columns
#   trimask [128, 128] f32  triu mask: m[tk, u] = 1 iff u >= tk
#   ident  [128, 128] f32r  identity (PE transpose)
#   ident2 [128, 64] f32r  eye(64) stacked twice (per-head transposes)
#   onesc  [1, 128]  f32r  ones row (K=1 broadcast matmuls)
# Output:
#   part   [T, D]   f32   this core's partial of y @ wo (normalized)
# --------------------------------------------------------------------------

def build_attn():
    nc = _bacc(NCORES)
    x_d = nc.dram_tensor("x", [T, D], F32, kind="ExternalInput")
    wqkv_d = nc.dram_tensor("wqkv", [D, 3 * CW], F32R, kind="ExternalInput")
    bqkv_d = nc.dram_tensor("bqkv", [3, CW], F32, kind="ExternalInput")
    wo_d = nc.dram_tensor("wo", [CW, D], F32R, kind="ExternalInput")
    trimask_d = nc.dram_tensor("trimask", [128, 128], F32, kind="ExternalInput")
    ident_d = nc.dram_tensor("ident", [128, 128], F32R, kind="ExternalInput")
    ident2_d = nc.dram_tensor("ident2", [128, 64], F32R, kind="ExternalInput")
    onesc_d = nc.dram_tensor("onesc", [1, 128], F32R, kind="ExternalInput")
    part_d = nc.dram_tensor("part", [T, D], F32, kind="ExternalOutput")

    NT = T // 128            # token tiles
    NJ = T // 512            # big token blocks
    NC = D // 128            # contraction chunks

    with tile.TileContext(nc, num_cores=NCORES) as tc:
        with (
            tc.tile_pool(name="const", bufs=1) as const,
            tc.tile_pool(name="xin", bufs=2) as xin,
            tc.tile_pool(name="stat", bufs=4) as stat,
            tc.tile_pool(name="ht", bufs=1) as htp,
            tc.tile_pool(name="qkv", bufs=1) as qkvp,
            tc.tile_pool(name="expp", bufs=6) as expp,
            tc.tile_pool(name="yout", bufs=4) as youtp,
            tc.tile_pool(name="ps", bufs=2, space="PSUM") as ps,
            tc.tile_pool(name="psy", bufs=2, space="PSUM") as psy,
        ):
            ident = const.tile([128, 128], F32R)
            nc.sync.dma_start(out=ident[:], in_=ident_d[:, :])
            ident2 = const.tile([128, 64], F32R)
            nc.sync.dma_start(out=ident2[:], in_=ident2_d[:, :])
            onesc = const.tile([1, 128], F32R)
            nc.sync.dma_start(out=onesc[:], in_=onesc_d[:, :])
            trimask = const.tile([128, 128], F32)
            nc.sync.dma_start(out=trimask[:], in_=trimask_d[:, :])
            wqkv = const.tile([128, NC, 3 * CW], F32R)
            nc.sync.dma_start(
                out=wqkv[:], in_=wqkv_d.ap().rearrange("(c p) m -> p c m", p=128))
            bqkv = const.tile([128, 3], F32)
            nc.sync.dma_start(
                out=bqkv[:], in_=bqkv_d.ap().rearrange("r m -> m r"))
            wo = const.tile([128, D], F32R)
            nc.sync.dma_start(out=wo[:], in_=wo_d[:, :])
            ones16 = const.tile([128, NT], F32)
            nc.vector.memset(ones16[:], 1.0)
            epst = const.tile([128, 1], F32)
            nc.vector.memset(epst[:], EPS)

            hT = htp.tile([128, NC, T], F32R)
            qT = qkvp.tile([128, T], F32R)
            kT = qkvp.tile([128, T], F32R)
            vT = qkvp.tile([128, T], F32R)
            vext = [youtp.tile([128, NT, HD + 1], F32R, name=f"vext{h}", bufs=1)
                    for h in range(HPC)]
            yT = qkvp.tile([128, T], F32R)
            dens = [qkvp.tile([1, T], F32R, name=f"den{h}") for h in range(HPC)]

            for h in range(HPC):
                nc.scalar.activation(out=vext[h][:, :, HD], in_=ones16[:],
                                     func=AF.Copy)

            def phase123(j):
                """rmsnorm + transpose + qkv projection + v transpose for
                token block j (512 tokens = 4 tiles)."""
                jsl = bass.ts(j, 512)
                for i in range(4 * j, 4 * j + 4):
                    xt = xin.tile([128, D], F32)
                    nc.sync.dma_start(out=xt[:],
                                      in_=x_d[i * 128:(i + 1) * 128, :])
                    ssum = stat.tile([128, 1], F32)
                    sq = stat.tile([128, D], F32, bufs=2)
                    nc.scalar.activation(out=sq[:], in_=xt[:], func=AF.Square,
                                         accum_out=ssum[:])
                    rstd = stat.tile([128, 1], F32)
                    nc.scalar.activation(out=rstd[:], in_=ssum[:],
                                         func=AF.Sqrt, scale=1.0 / D,
                                         bias=epst[:])
                    nc.vector.reciprocal(out=rstd[:], in_=rstd[:])
                    h = xin.tile([128, D], F32R)
                    nc.vector.tensor_scalar_mul(h[:], xt[:], rstd[:])
                    for c2 in range(0, NC, 4):
                        ptr = ps.tile([128, 512], F32R, tag="tr")
                        for c in range(c2, c2 + 4):
                            nc.tensor.transpose(
                                ptr[:, (c - c2) * 128:(c - c2 + 1) * 128],
                                h[:, c * 128:(c + 1) * 128], ident[:])
                        dst = hT[:, c2:c2 + 4, i * 128:(i + 1) * 128]
                        src = ptr[:].rearrange("p (c q) -> p c q", q=128)
                        if (i + c2) % 2 == 0:
                            nc.scalar.copy(dst, src)
                        else:
                            nc.vector.tensor_copy(dst, src)
                # qkv projections for this block
                for out_t, col0, brow in ((qT, 0, 0), (kT, CW, 1),
                                          (vT, 2 * CW, 2)):
                    pmm = psy.tile([128, 512], F32, tag="yacc")
                    for c in range(NC):
                        nc.tensor.matmul(
                            pmm[:], wqkv[:, c, col0:col0 + CW],
                            hT[:, c, jsl], start=(c == 0), stop=(c == NC - 1))
                    if brow < 2:
                        nc.vector.tensor_scalar_add(out_t[:, jsl], pmm[:],
                                                    bqkv[:, brow:brow + 1])
                    else:
                        nc.scalar.activation(out=out_t[:, jsl], in_=pmm[:],
                                             func=AF.Identity,
                                             bias=bqkv[:, brow:brow + 1])
                # v transposes for this block (grouped per head: PE wedges on
                # mixed-base transposes within one psum tile)
                for h in range(HPC):
                    ptr = ps.tile([128, 256], F32R, tag="trv")
                    for i in range(4 * j, 4 * j + 4):
                        slot = (i - 4 * j) * 64
                        nc.tensor.transpose(
                            ptr[:, slot:slot + 64],
                            vT[h * HD:(h + 1) * HD, i * 128:(i + 1) * 128],
                            ident2[h * HD:(h + 1) * HD, :])
                    nc.vector.tensor_copy(
                        vext[h][:, 4 * j:4 * j + 4, 0:HD],
                        ptr[:].rearrange("p (i d) -> p i d", d=64))

            def attention(jq):
                """causal attention for query block jq, both heads."""
                jsl = bass.ts(jq, 512)
                for h in range(HPC):
                    hsl = slice(h * HD, (h + 1) * HD)
                    pacc = psy.tile([128, 512], F32, tag="yacc")
                    nblk = 4 * jq + 4
                    for ib in range(nblk):
                        off = (ib - 4 * jq) * 128 if ib >= 4 * jq else 0
                        pss = ps.tile([128, 512], F32, tag="scores")
                        nc.tensor.matmul(
                            pss[:, off:512],
                            kT[hsl, ib * 128:(ib + 1) * 128],
                            qT[hsl, jsl][:, off:512], start=True, stop=True)
                        et = expp.tile([128, 512], F32R, tag="exp")
                        nc.scalar.activation(out=et[:, off:512],
                                             in_=pss[:, off:512],
                                             func=AF.Exp,
                                             scale=1.0 / math.sqrt(HD))
                        if ib >= 4 * jq:
                            # triangular boundary strip
                            nc.vector.tensor_mul(et[:, off:off + 128],
                                                 et[:, off:off + 128],
                                                 trimask[:])
                        nc.tensor.matmul(
                            pacc[0:HD + 1, off:512], vext[h][:, ib, :],
                            et[:, off:512],
                            start=(ib == 0), stop=(ib == nblk - 1))
                    if h == 0:
                        nc.scalar.copy(yT[hsl, jsl], pacc[0:HD, :])
                    else:
                        nc.vector.tensor_copy(yT[hsl, jsl], pacc[0:HD, :])
                    nc.vector.tensor_copy(dens[h][0:1, jsl],
                                          pacc[HD:HD + 1, :])

            for j in range(NJ):
                phase123(j)
                attention(j)

            # ---- normalize yT by denominators ----
            with nc.allow_low_precision(
                    reason="f32r rounding of softmax denominator "
                           "reciprocals (~2^-11) is negligible"):
                for h in range(HPC):
                    nc.vector.reciprocal(out=dens[h][:], in_=dens[h][:])
            for h in range(HPC):
                hsl = slice(h * HD, (h + 1) * HD)
                for j in range(NJ):
                    jsl = bass.ts(j, 512)
                    pbd = psy.tile([128, 512], F32, tag="yacc")
                    nc.tensor.matmul(pbd[:], onesc[:], dens[h][0:1, jsl],
                                     start=True, stop=True)
                    nc.vector.tensor_mul(yT[hsl, jsl], yT[hsl, jsl],
                                         pbd[hsl, :])

            # ---- partial output projection: part = yTn.T @ wo ----
            for i in range(NT):
                for half in range(2):
                    pso = psy.tile([128, 512], F32, tag="yacc")
                    nc.tensor.matmul(
                        pso[:], yT[:, i * 128:(i + 1) * 128],
                        wo[:, half * 512:(half + 1) * 512],
                        start=True, stop=True)
                    ot = youtp.tile([128, 512], F32, tag="out")
                    if (i + half) % 2 == 0:
                        nc.scalar.copy(ot[:], pso[:])
                    else:
                        nc.vector.tensor_copy(ot[:], pso[:])
                    nc.sync.dma_start(
                        out=part_d[i * 128:(i + 1) * 128,
                                   half * 512:(half + 1) * 512],
                        in_=ot[:])
    nc.compile()
    return nc


# --------------------------------------------------------------------------
# Launch B: one expert per core (bf16 matmuls, fp32 accumulation).
# Per-core inputs:
#   tokT [D, CAP]  bf16  gathered+normed tokens (transposed), zero-padded
#   gu   [D, 2H]   bf16  [gate | up] for this core's expert
#   down [H, D]    bf16  down projection
#   wts  [CAP/128, 128] f32  routing weight * MOE_SCALE per slot (0 for pads)
# Output:
#   eout [CAP, D]  f32   weighted expert output per slot
# --------------------------------------------------------------------------

def build_moe():
    nc = _bacc(NCORES)
    tokT_d = nc.dram_tensor("tokT", [D, CAP], BF16, kind="ExternalInput")
    gu_d = nc.dram_tensor("gu", [D, 2 * H], BF16, kind="ExternalInput")
    down_d = nc.dram_tensor("down", [H, D], BF16, kind="ExternalInput")
    wts_d = nc.dram_tensor("wts", [CAP // 128, 128], F32, kind="ExternalInput")
    eout_d = nc.dram_tensor("eout", [CAP, D], F32, kind="ExternalOutput")

    NC = D // 128            # 8 d chunks
    NHT = H // 128           # 16 h tiles
    NTT = CAP // 128         # 5 token tiles

    with tile.TileContext(nc, num_cores=NCORES) as tc:
        with (
            tc.tile_pool(name="const", bufs=1) as const,
            tc.tile_pool(name="wstream", bufs=3) as wstream,
            tc.tile_pool(name="gup", bufs=1) as gup,
            tc.tile_pool(name="outp", bufs=4) as outp,
            tc.tile_pool(name="ps", bufs=2, space="PSUM") as ps,
            tc.tile_pool(name="psu", bufs=2, space="PSUM") as psu,
        ):
            tokT = const.tile([128, NC, CAP], BF16)
            nc.sync.dma_start(
                out=tokT[:], in_=tokT_d.ap().rearrange("(c p) n -> p c n", p=128))
            wts = const.tile([128, NTT], F32)
            nc.sync.dma_start(out=wts[:], in_=wts_d.ap().rearrange("t p -> p t"))
            down = const.tile([128, NHT, D], BF16)
            nc.sync.dma_start(
                out=down[:], in_=down_d.ap().rearrange("(t p) m -> p t m", p=128))

            guT = gup.tile([128, NHT, CAP], BF16)
            for t in range(NHT):
                gw = wstream.tile([128, NC, 128], BF16, tag="gw")
                nc.sync.dma_start(
                    out=gw[:],
                    in_=gu_d.ap()[:, t * 128:(t + 1) * 128]
                    .rearrange("(c p) m -> p c m", p=128))
                uw = wstream.tile([128, NC, 128], BF16, tag="uw")
                nc.sync.dma_start(
                    out=uw[:],
                    in_=gu_d.ap()[:, H + t * 128:H + (t + 1) * 128]
                    .rearrange("(c p) m -> p c m", p=128))
                for n0, n1 in ((0, 512), (512, CAP)):
                    psg = ps.tile([128, 512], F32, tag="g")
                    psuu = psu.tile([128, 512], F32, tag="u")
                    nw = n1 - n0
                    for c in range(NC):
                        nc.tensor.matmul(psg[:, 0:nw], gw[:, c, :],
                                         tokT[:, c, n0:n1],
                                         start=(c == 0), stop=(c == NC - 1))
                    for c in range(NC):
                        nc.tensor.matmul(psuu[:, 0:nw], uw[:, c, :],
                                         tokT[:, c, n0:n1],
                                         start=(c == 0), stop=(c == NC - 1))
                    sg = outp.tile([128, 512], F32, tag="sg")
                    if SIM_COMPAT:
                        nc.scalar.activation(out=sg[:, 0:nw], in_=psg[:, 0:nw],
                                             func=AF.Sigmoid)
                        nc.vector.tensor_mul(sg[:, 0:nw], sg[:, 0:nw],
                                             psg[:, 0:nw])
                    else:
                        nc.scalar.activation(out=sg[:, 0:nw], in_=psg[:, 0:nw],
                                             func=AF.Silu)
                    nc.vector.tensor_mul(guT[:, t, n0:n1], sg[:, 0:nw],
                                         psuu[:, 0:nw])

            for tt in range(NTT):
                for half in range(2):
                    pso = ps.tile([128, 512], F32, tag="o")
                    for t in range(NHT):
                        nc.tensor.matmul(
                            pso[:], guT[:, t, tt * 128:(tt + 1) * 128],
                            down[:, t, half * 512:(half + 1) * 512],
                            start=(t == 0), stop=(t == NHT - 1))
                    ot = outp.tile([128, 512], F32, tag="ot")
                    nc.vector.tensor_scalar_mul(ot[:], pso[:],
                                                wts[:, tt:tt + 1])
                    nc.sync.dma_start(
                        out=eout_d[tt * 128:(tt + 1) * 128,
                                   half * 512:(half + 1) * 512],
                        in_=ot[:])
    nc.compile()
    return nc


# --------------------------------------------------------------------------
# Host orchestration
# --------------------------------------------------------------------------

def _get(name, builder):
    if name not in _CACHE:
        _CACHE[name] = builder()
    return _CACHE[name]


def _attn_inputs(x2d, wq, bq, wkv, bkv, wo, norm1_w):
    """Build the 8 per-core input maps for launch A."""
    # fold norm1_w into the projection rows
    wq_s = wq * norm1_w[:, None]
    wkv_s = wkv * norm1_w[:, None]
    wk_s = wkv_s[:, :D]
    wv_s = wkv_s[:, D:]
    bk = bkv[:D]
    bv = bkv[D:]

    tk = np.arange(128)[:, None]
    u = np.arange(128)[None, :]
    trimask = (u >= tk).astype(np.float32)
    ident = np.eye(128, dtype=np.float32)
    ident2 = np.concatenate([np.eye(64, dtype=np.float32)] * 2, axis=0)
    onesc = np.ones((1, 128), np.float32)

    ins = []
    for c in range(NCORES):
        cs = slice(c * CW, (c + 1) * CW)
        wqkv_c = np.ascontiguousarray(
            np.concatenate([wq_s[:, cs], wk_s[:, cs], wv_s[:, cs]], axis=1))
        bqkv_c = np.ascontiguousarray(
            np.stack([bq[cs], bk[cs], bv[cs]], axis=0))
        wo_c = np.ascontiguousarray(wo[cs, :])
        ins.append({
            "x": x2d,
            "wqkv": wqkv_c,
            "bqkv": bqkv_c,
            "wo": wo_c,
            "trimask": trimask,
            "ident": ident,
            "ident2": ident2,
            "onesc": onesc,
        })
    return ins


def _route(x2, router_w, norm2_w):
    """Exact reference routing on host: rmsnorm2 + top-2 + softmax."""
    h2 = x2 / np.sqrt(np.mean(x2 * x2, axis=-1, keepdims=True) + EPS)
    h2 = (h2 * norm2_w).astype(np.float32)
    logits = h2.astype(np.float32) @ router_w.astype(np.float32)   # [N, E]
    idx1 = np.argmax(logits, axis=-1)
    l2 = logits.copy()
    l2[np.arange(T), idx1] = -np.inf
    idx2 = np.argmax(l2, axis=-1)
    v1 = logits[np.arange(T), idx1]
    v2 = logits[np.arange(T), idx2]
    # softmax over the two selected logits (v1 >= v2)
    e2 = np.exp((v2 - v1).astype(np.float32))
    p1 = (1.0 / (1.0 + e2)).astype(np.float32)
    p2 = (e2 / (1.0 + e2)).astype(np.float32)
    return h2, idx1, idx2, p1, p2


def kernel(x, freqs_cos, freqs_sin, norm1_w, wq, bq, wkv, bkv, wo, bo,
           norm2_w, router_w, gate_w, up_w, down_w):
    x = np.asarray(x, np.float32)
    x2d = np.ascontiguousarray(x.reshape(T, D))
    wq = np.asarray(wq, np.float32)
    wkv = np.asarray(wkv, np.float32)
    wo = np.asarray(wo, np.float32)
    bq = np.asarray(bq, np.float32)
    bkv = np.asarray(bkv, np.float32)
    bo = np.asarray(bo, np.float32)
    norm1_w = np.asarray(norm1_w, np.float32)
    norm2_w = np.asarray(norm2_w, np.float32)
    router_w = np.asarray(router_w, np.float32)
    gate_w = np.asarray(gate_w, np.float32)
    up_w = np.asarray(up_w, np.float32)
    down_w = np.asarray(down_w, np.float32)

    # ---- launch A ----
    nc_a = _get("attn", build_attn)
    ins_a = _attn_inputs(x2d, wq, bq, wkv, bkv, wo, norm1_w)
    res_a = run_bass_kernel_spmd(nc_a, ins_a, core_ids=list(range(NCORES)))
    parts = np.stack([res_a.results[c]["part"] for c in range(NCORES)])
    x2 = (x2d.astype(np.float64) + parts.sum(axis=0, dtype=np.float64)
          + bo.astype(np.float64)).astype(np.float32)

    # ---- host routing ----
    h2, idx1, idx2, p1, p2 = _route(x2, router_w, norm2_w)

    # per-expert token lists (order: top-1 hits then top-2 hits, stable)
    work = []   # (expert, token_idx array, weight array)
    for e in range(E):
        m1 = idx1 == e
        m2 = idx2 == e
        toks = np.concatenate([np.nonzero(m1)[0], np.nonzero(m2)[0]])
        wgts = np.concatenate([p1[m1], p2[m2]]).astype(np.float32)
        for s in range(0, len(toks), CAP):
            work.append((e, toks[s:s + CAP], wgts[s:s + CAP]))

    h2b = h2.astype(BF16_NP)
    gub: dict = {}
    downb: dict = {}

    # ---- launch B (usually one round of 8) ----
    nc_b = _get("moe", build_moe)
    moe = np.zeros((T, D), np.float64)
    for r0 in range(0, len(work), NCORES):
        batch = work[r0:r0 + NCORES]
        while len(batch) < NCORES:
            batch.append((0, np.zeros(0, np.int64), np.zeros(0, np.float32)))
        ins_b = []
        for e, toks, wgts in batch:
            tokT = np.zeros((D, CAP), BF16_NP)
            tokT[:, :len(toks)] = h2b[toks].T
            wts = np.zeros((CAP,), np.float32)
            wts[:len(toks)] = wgts * MOE_SCALE
            if e not in gub:
                gub[e] = np.ascontiguousarray(np.concatenate(
                    [gate_w[e], up_w[e]], axis=1).astype(BF16_NP))
                downb[e] = np.ascontiguousarray(down_w[e].astype(BF16_NP))
            ins_b.append({
                "tokT": tokT,
                "gu": gub[e],
                "down": downb[e],
                "wts": np.ascontiguousarray(wts.reshape(CAP // 128, 128)),
            })
        res_b = run_bass_kernel_spmd(nc_b, ins_b, core_ids=list(range(NCORES)))
        for (e, toks, wgts), rc in zip(batch, res_b.results):
            if len(toks):
                moe[toks] += rc["eout"][:len(toks)].astype(np.float64)

    out = (x2.astype(np.float64) + moe).astype(np.float32)
    return out.reshape(B, T, D)
